# revision 24
# baseline (speedup 1.0000x reference)
"""CrossAttention GTrXL kernel for 8 Trainium2 NeuronCores.

Sharding: tensor-parallel over heads. 16 heads / 8 cores = 2 heads per core.
Each core:
  - computes q/k/v projections for its 2 heads (128 of the 1024 hidden dims),
  - runs attention (scores -> exp -> attn@val) for its 4 (batch, head) instances,
  - normalizes by the softmax denominator (carried as a ones-column in val),
  - AllGathers the per-head attention vectors (bf16, in 4 slices so the
    collectives overlap compute),
  - computes a 128-column slice of the output projection.
Host side: transpose + bf16-cast inputs, shard weights, reassemble output.

Phase overlap: the batch-1 projections are emitted as "filler" PE work inside
the batch-0 attention loop (which is ScalarE/exp-bound), and the batch-0
output projection fills the batch-1 attention loop. This keeps TensorE busy
(HAM stays un-throttled) and hides the AllGather latency.
"""

from collections import deque

import numpy as np
import ml_dtypes

import concourse.bass as bass
import concourse.mybir as mybir
import concourse.tile as tile
from concourse import bacc
from concourse.bass_utils import run_bass_kernel_spmd

BF16 = mybir.dt.bfloat16
F32 = mybir.dt.float32

S = 2048   # dec seq
T = 2048   # enc seq
B = 2
E = 1024
H = 16
D = 64
HD = H * D           # 1024
N_CORES = 8
H_LOC = H // N_CORES  # 2 heads per core
PD = H_LOC * D        # 128 partition dims per core
BS = B * S            # 4096
BT = B * T            # 4096
SCALE = 1.0 / D ** 0.5

KT = E // 128        # 8 contraction tiles for projections
TTI = T // 128       # 16 t-tiles per batch
VBLK = 2 * 65        # val block per t-tile: 2 heads x (ones col + 64 vals)
SW = 2               # s windows per batch (1024 wide)
SWW = S // SW        # 1024


def build_program():
    nc = bacc.Bacc("TRN2", target_bir_lowering=False, debug=False,
                   num_devices=N_CORES)

    # ---- I/O -----------------------------------------------------------
    xT = nc.dram_tensor("xT", [E, BS], BF16, kind="ExternalInput")       # inputs^T, col = b*S+s
    eT = nc.dram_tensor("eT", [E, BT], BF16, kind="ExternalInput")       # enc^T, col = b*T+t
    wq = nc.dram_tensor("wq", [E, PD], BF16, kind="ExternalInput")
    wk = nc.dram_tensor("wk", [E, PD], BF16, kind="ExternalInput")
    wv = nc.dram_tensor("wv", [E, PD], BF16, kind="ExternalInput")
    wp = nc.dram_tensor("wp", [HD, PD], BF16, kind="ExternalInput")      # full rows, my E-cols
    qbias = nc.dram_tensor("qbias", [PD, 1], F32, kind="ExternalInput")  # u + bq (per hd)
    kbias = nc.dram_tensor("kbias", [PD, 1], F32, kind="ExternalInput")  # bkv k-part
    bpcol = nc.dram_tensor("bpcol", [PD, 1], F32, kind="ExternalInput")  # bp slice col
    out = nc.dram_tensor("out", [PD, BS], F32, kind="ExternalOutput")    # out^T: [my E cols, b*S+s]

    with tile.TileContext(nc) as tc:
        with tc.tile_pool(name="persist", bufs=1) as persist, \
             tc.tile_pool(name="dram", bufs=1, space="DRAM") as dram, \
             tc.tile_pool(name="psum", bufs=2, space="PSUM") as psum, \
             tc.tile_pool(name="att", bufs=3) as att, \
             tc.tile_pool(name="nrm", bufs=2) as nrm, \
             tc.tile_pool(name="avs", bufs=2) as avs, \
             tc.tile_pool(name="agp", bufs=12) as agp, \
             tc.tile_pool(name="otp", bufs=4) as otp:

            # persistent SBUF tensors
            qT_b = [persist.tile([PD, S], BF16, tag=f"qT{b}", name=f"qT{b}") for b in range(B)]
            kT_b = [persist.tile([PD, T], BF16, tag=f"kT{b}", name=f"kT{b}") for b in range(B)]
            val_b = [persist.tile([128, TTI * VBLK], BF16, tag=f"val{b}", name=f"val{b}")
                     for b in range(B)]
            qb_sb = persist.tile([PD, 1], F32)
            kb_sb = persist.tile([PD, 1], F32)
            bp_sb = persist.tile([PD, 1], F32)
            wq_sb = persist.tile([128, KT * PD], BF16)  # k-tiles side by side
            wk_sb = persist.tile([128, KT * PD], BF16)
            wv_sb = persist.tile([128, KT * PD], BF16)
            wp_sb = persist.tile([128, KT * PD], BF16)

            for b in range(B):
                nc.vector.memset(val_b[b][:], 1.0)  # ones cols survive copies
            nc.sync.dma_start(out=qb_sb[:], in_=qbias.ap())
            nc.sync.dma_start(out=kb_sb[:], in_=kbias.ap())
            nc.sync.dma_start(out=bp_sb[:], in_=bpcol.ap())
            for k in range(KT):
                nc.sync.dma_start(out=wk_sb[:, k * PD:(k + 1) * PD],
                                  in_=wk[k * 128:(k + 1) * 128, :])
                nc.sync.dma_start(out=wv_sb[:, k * PD:(k + 1) * PD],
                                  in_=wv[k * 128:(k + 1) * 128, :])
                nc.sync.dma_start(out=wq_sb[:, k * PD:(k + 1) * PD],
                                  in_=wq[k * 128:(k + 1) * 128, :])
                nc.sync.dma_start(out=wp_sb[:, k * PD:(k + 1) * PD],
                                  in_=wp[k * 128:(k + 1) * 128, :])

            # input tiles, split per batch so batch-0 work starts earliest:
            # load order et_b0, xt_b0, et_b1, xt_b1
            eT_t = [[None] * KT for _ in range(B)]
            xT_t = [[None] * KT for _ in range(B)]
            qeng = [nc.sync, nc.gpsimd, nc.scalar]
            for b in range(B):
                for k in range(KT):
                    et = persist.tile([128, T], BF16, tag=f"et{b}_{k}",
                                      name=f"et{b}_{k}")
                    qeng[k % 3].dma_start(
                        out=et[:], in_=eT[k * 128:(k + 1) * 128,
                                          b * T:(b + 1) * T])
                    eT_t[b][k] = et
                for k in range(KT):
                    xt = persist.tile([128, S], BF16, tag=f"xt{b}_{k}",
                                      name=f"xt{b}_{k}")
                    qeng[k % 3].dma_start(
                        out=xt[:], in_=xT[k * 128:(k + 1) * 128,
                                          b * S:(b + 1) * S])
                    xT_t[b][k] = xt

            # DRAM bounce + AllGather buffers, one per (b, s-window)
            av_dram = [[dram.tile([PD, SWW], BF16, tag=f"avd{b}{sw}", name=f"avd{b}{sw}")
                        for sw in range(SW)] for b in range(B)]
            ag_dram = [[dram.tile([HD, SWW], BF16, tag=f"agd{b}{sw}", name=f"agd{b}{sw}",
                                  addr_space="Shared")
                        for sw in range(SW)] for b in range(B)]

            # ---------------- unit generators --------------------------
            def proj_chunk_unit(b, ch, which):
                """512-wide chunk of the qT / kT projection for batch b."""
                def emit():
                    src = xT_t[b] if which == "q" else eT_t[b]
                    w = wq_sb if which == "q" else wk_sb
                    bias = qb_sb if which == "q" else kb_sb
                    dst = qT_b[b] if which == "q" else kT_b[b]
                    p = psum.tile([PD, 512], F32, tag="ps", name="pproj")
                    for k in range(KT):
                        nc.tensor.matmul(p[:], w[:, k * PD:(k + 1) * PD],
                                         src[k][:, ch * 512:(ch + 1) * 512],
                                         start=(k == 0), stop=(k == KT - 1))
                    nc.vector.tensor_scalar_add(
                        dst[:, ch * 512:(ch + 1) * 512], p[:], bias[:])
                return emit

            def val_unit(b, ti):
                """One 128-row t-tile of the val projection for batch b."""
                def emit():
                    p = psum.tile([128, PD], F32, tag="ps", name="pval")
                    for k in range(KT):
                        nc.tensor.matmul(p[:],
                                         eT_t[b][k][:, ti * 128:(ti + 1) * 128],
                                         wv_sb[:, k * PD:(k + 1) * PD],
                                         start=(k == 0), stop=(k == KT - 1))
                    for h in range(H_LOC):
                        # values at block offsets 0..63, ones col at 64
                        nc.vector.tensor_copy(
                            val_b[b][:, ti * VBLK + h * 65:
                                    ti * VBLK + h * 65 + 64],
                            p[:, h * 64:(h + 1) * 64])
                return emit

            def proj_units(b):
                units = []
                for ch in range(4):
                    units.append(proj_chunk_unit(b, ch, "k"))
                for ti in range(TTI):
                    units.append(val_unit(b, ti))
                for ch in range(4):
                    units.append(proj_chunk_unit(b, ch, "q"))
                return units

            def outproj_units(b, sw):
                """Transposed output projection, out^T[e, bs] slice for
                columns b*S + sw*SWW .. +SWW, in two 512-wide chunks."""
                units = []
                ag_tiles = {}

                def load_unit(ch):
                    def emit():
                        for k in range(KT):
                            a = agp.tile([128, 512], BF16, tag="ag", name="ag")
                            nc.sync.dma_start(
                                out=a[:],
                                in_=ag_dram[b][sw][k * 128:(k + 1) * 128,
                                                   ch * 512:(ch + 1) * 512])
                            ag_tiles[(ch, k)] = a
                    return emit

                def mm_unit(ch):
                    def emit():
                        po = psum.tile([128, 512], F32, tag="ps", name="po")
                        for k in range(KT):
                            nc.tensor.matmul(
                                po[:], wp_sb[:, k * PD:(k + 1) * PD],
                                ag_tiles[(ch, k)][:],
                                start=(k == 0), stop=(k == KT - 1))
                        o = otp.tile([128, 512], F32, tag="o", name="o")
                        nc.vector.tensor_scalar_add(o[:], po[:], bp_sb[:])
                        col = b * S + sw * SWW + ch * 512
                        nc.sync.dma_start(out=out[:, col:col + 512], in_=o[:])
                    return emit
                for ch in range(2):
                    units.append(load_unit(ch))
                    units.append(mm_unit(ch))
                return units

            # ---------------- attention for one batch ------------------
            def attention(b, filler, fill_from=0, early_double=0):
                """Attention for batch b; pops one filler unit per (sw, ti)
                iteration index >= fill_from (two per iteration for the first
                `early_double` iterations) to keep TensorE busy."""
                it = 0
                for sw in range(SW):
                    avp = [psum.tile([65, SWW], F32, tag="av", name="avp") for _ in range(H_LOC)]
                    for ti in range(TTI):
                        psc = [psum.tile([128, SWW], F32, tag="ps", name="psc")
                               for _ in range(H_LOC)]
                        # scores: interleave heads so the K=64 matmuls pair up
                        # in the PE array (row groups 0-63 / 64-127)
                        for n in range(2):
                            for h in range(H_LOC):
                                hsl = slice(h * 64, (h + 1) * 64)
                                nc.tensor.matmul(
                                    psc[h][:, n * 512:(n + 1) * 512],
                                    kT_b[b][hsl, ti * 128:(ti + 1) * 128],
                                    qT_b[b][hsl, sw * SWW + n * 512:
                                            sw * SWW + (n + 1) * 512],
                                    start=True, stop=True)
                        for h in range(H_LOC):
                            p_sb = att.tile([128, SWW], BF16, tag="p", name="p_sb")
                            nc.scalar.activation(
                                p_sb[:], psc[h][:],
                                mybir.ActivationFunctionType.Exp, scale=SCALE)
                            vblk = val_b[b][:, ti * VBLK + h * 65:
                                            ti * VBLK + h * 65 + 65]
                            for n in range(2):
                                nc.tensor.matmul(
                                    avp[h][:, n * 512:(n + 1) * 512],
                                    vblk, p_sb[:, n * 512:(n + 1) * 512],
                                    start=(ti == 0), stop=(ti == TTI - 1))
                        if filler and it >= fill_from and \
                                (it >= 10 or it % 2 == 0):
                            filler.popleft()()
                            if filler and it < early_double:
                                filler.popleft()()
                        it += 1
                    # normalize + evict this s-window (sumexp in row 64)
                    av_st = avs.tile([D, 2 * SWW], BF16, tag="avst", name="av_st")
                    for h in range(H_LOC):
                        z = nrm.tile([1, SWW], F32, tag="zrb", name="z")
                        nc.vector.tensor_copy(z[0:1, :], avp[h][64:65, :])
                        zr = nrm.tile([1, SWW], F32, tag="zrb", name="zr")
                        nc.vector.reciprocal_approx_fast(zr[0:1, :], z[0:1, :])
                        rb = nrm.tile([D, SWW], F32, tag="zrb", name="rb")
                        nc.gpsimd.partition_broadcast(rb[:], zr[0:1, :])
                        nc.vector.tensor_mul(
                            av_st[:, h * SWW:(h + 1) * SWW],
                            avp[h][0:64, :], rb[:])
                    nc.sync.dma_start(
                        out=av_dram[b][sw].rearrange("(h d) s -> d h s", h=H_LOC),
                        in_=av_st.rearrange("d (h s) -> d h s", h=H_LOC))
                    nc.gpsimd.collective_compute(
                        "AllGather", mybir.AluOpType.bypass,
                        replica_groups=[list(range(N_CORES))],
                        ins=[av_dram[b][sw].opt()], outs=[ag_dram[b][sw].opt()])

            # ---------------- schedule ---------------------------------
            # prefix: all enc-dependent b0 projections run while the x/enc
            # DMAs stream in, then the first two qT chunks
            prefix = [proj_chunk_unit(0, ch, "k") for ch in range(4)] + \
                     [val_unit(0, ti) for ti in range(TTI)] + \
                     [proj_chunk_unit(0, 0, "q"), proj_chunk_unit(0, 1, "q")]
            for u in prefix:
                u()
            fill0 = deque(
                [proj_chunk_unit(0, 2, "q"), proj_chunk_unit(0, 3, "q")] +
                [proj_chunk_unit(1, ch, "k") for ch in range(4)] +
                [val_unit(1, ti) for ti in range(TTI)] +
                [proj_chunk_unit(1, ch, "q") for ch in range(4)])
            attention(0, fill0, fill_from=0)
            for u in fill0:
                u()
            fill1 = deque(outproj_units(0, 0) + outproj_units(0, 1))
            attention(1, fill1, fill_from=8)
            for u in fill1:
                u()
            # tail: outproj b1/sw0 right away, dummies keep PE warm while the
            # last AllGather finishes, then outproj b1/sw1
            for u in outproj_units(1, 0):
                u()
            for i in range(16):
                proj_chunk_unit(0, i % 4, "q")()
            for u in outproj_units(1, 1):
                u()

    nc.compile()
    return nc


_NC_CACHE = None


def _get_program():
    global _NC_CACHE
    if _NC_CACHE is None:
        _NC_CACHE = build_program()
    return _NC_CACHE


def _make_in_maps(inputs, pos_embedding, encoder_hidden_states, u, v, mask,
                  Wkv, bkv, Wq, bq, Wp, bp):
    bf = ml_dtypes.bfloat16
    xT = np.ascontiguousarray(
        np.asarray(inputs, np.float32).transpose(2, 1, 0).reshape(E, BS)).astype(bf)
    eT = np.ascontiguousarray(
        np.asarray(encoder_hidden_states, np.float32).transpose(2, 1, 0)
        .reshape(E, BT)).astype(bf)
    Wkv = np.asarray(Wkv, np.float32)
    Wq = np.asarray(Wq, np.float32)
    Wp = np.asarray(Wp, np.float32)
    bkv = np.asarray(bkv, np.float32)
    bq = np.asarray(bq, np.float32)
    bp = np.asarray(bp, np.float32)
    uf = np.asarray(u, np.float32).reshape(HD)
    in_maps = []
    for c in range(N_CORES):
        sl = slice(c * PD, (c + 1) * PD)
        in_maps.append({
            "xT": xT,
            "eT": eT,
            "wq": np.ascontiguousarray(Wq[:, sl]).astype(bf),
            "wk": np.ascontiguousarray(Wkv[:, sl]).astype(bf),
            "wv": np.ascontiguousarray(Wkv[:, HD + c * PD: HD + (c + 1) * PD]).astype(bf),
            "wp": np.ascontiguousarray(Wp[:, sl]).astype(bf),
            "qbias": (uf[sl] + bq[sl]).reshape(PD, 1).astype(np.float32),
            "kbias": bkv[sl].reshape(PD, 1).astype(np.float32),
            "bpcol": (bp[sl] + bkv[HD:] @ Wp[:, sl]).reshape(PD, 1)
                     .astype(np.float32),
        })
    return in_maps


def _assemble(results):
    full = np.empty((S, B, E), np.float32)
    for c in range(N_CORES):
        part = np.asarray(results[c]["out"]).reshape(PD, B, S)
        full[:, :, c * PD:(c + 1) * PD] = part.transpose(2, 1, 0)
    return full


def run(trace=False, **inputs):
    nc = _get_program()
    in_maps = _make_in_maps(**inputs)
    res = run_bass_kernel_spmd(nc, in_maps, core_ids=list(range(N_CORES)),
                               trace=trace)
    return _assemble(res.results), res


def kernel(**inputs):
    out, _ = run(**inputs)
    return out


# revision 25
# speedup vs baseline: 1.0297x; 1.0297x over previous
"""CrossAttention GTrXL kernel for 8 Trainium2 NeuronCores.

Sharding: tensor-parallel over heads. 16 heads / 8 cores = 2 heads per core.
Each core:
  - computes q/k/v projections for its 2 heads (128 of the 1024 hidden dims),
  - runs attention (scores -> exp -> attn@val) for its 4 (batch, head) instances,
  - normalizes by the softmax denominator (carried as a ones-column in val),
  - AllGathers the per-head attention vectors (bf16, in 4 slices so the
    collectives overlap compute),
  - computes a 128-column slice of the output projection.
Host side: transpose + bf16-cast inputs, shard weights, reassemble output.

Phase overlap: the batch-1 projections are emitted as "filler" PE work inside
the batch-0 attention loop (which is ScalarE/exp-bound), and the batch-0
output projection fills the batch-1 attention loop. This keeps TensorE busy
(HAM stays un-throttled) and hides the AllGather latency.
"""

from collections import deque

import numpy as np
import ml_dtypes

import concourse.bass as bass
import concourse.mybir as mybir
import concourse.tile as tile
from concourse import bacc
from concourse.bass_utils import run_bass_kernel_spmd

BF16 = mybir.dt.bfloat16
F32 = mybir.dt.float32

S = 2048   # dec seq
T = 2048   # enc seq
B = 2
E = 1024
H = 16
D = 64
HD = H * D           # 1024
N_CORES = 8
H_LOC = H // N_CORES  # 2 heads per core
PD = H_LOC * D        # 128 partition dims per core
BS = B * S            # 4096
BT = B * T            # 4096
SCALE = 1.0 / D ** 0.5

KT = E // 128        # 8 contraction tiles for projections
TTI = T // 128       # 16 t-tiles per batch
VBLK = 2 * 65        # val block per t-tile: 2 heads x (ones col + 64 vals)
SW = 2               # s windows per batch (1024 wide)
SWW = S // SW        # 1024


def build_program():
    nc = bacc.Bacc("TRN2", target_bir_lowering=False, debug=False,
                   num_devices=N_CORES)

    # ---- I/O -----------------------------------------------------------
    xT = nc.dram_tensor("xT", [E, BS], BF16, kind="ExternalInput")       # inputs^T, col = b*S+s
    eT = nc.dram_tensor("eT", [E, BT], BF16, kind="ExternalInput")       # enc^T, col = b*T+t
    wq = nc.dram_tensor("wq", [E, PD], BF16, kind="ExternalInput")
    wk = nc.dram_tensor("wk", [E, PD], BF16, kind="ExternalInput")
    wv = nc.dram_tensor("wv", [E, PD], BF16, kind="ExternalInput")
    wp = nc.dram_tensor("wp", [HD, PD], BF16, kind="ExternalInput")      # full rows, my E-cols
    qbias = nc.dram_tensor("qbias", [PD, 1], F32, kind="ExternalInput")  # u + bq (per hd)
    kbias = nc.dram_tensor("kbias", [PD, 1], F32, kind="ExternalInput")  # bkv k-part
    bpcol = nc.dram_tensor("bpcol", [PD, 1], F32, kind="ExternalInput")  # bp slice col
    out = nc.dram_tensor("out", [PD, BS], F32, kind="ExternalOutput")    # out^T: [my E cols, b*S+s]

    with tile.TileContext(nc) as tc:
        with tc.tile_pool(name="persist", bufs=1) as persist, \
             tc.tile_pool(name="dram", bufs=1, space="DRAM") as dram, \
             tc.tile_pool(name="psum", bufs=2, space="PSUM") as psum, \
             tc.tile_pool(name="att", bufs=3) as att, \
             tc.tile_pool(name="nrm", bufs=2) as nrm, \
             tc.tile_pool(name="avs", bufs=2) as avs, \
             tc.tile_pool(name="agp", bufs=12) as agp, \
             tc.tile_pool(name="otp", bufs=4) as otp:

            # persistent SBUF tensors
            qT_b = [persist.tile([PD, S], BF16, tag=f"qT{b}", name=f"qT{b}") for b in range(B)]
            kT_b = [persist.tile([PD, T], BF16, tag=f"kT{b}", name=f"kT{b}") for b in range(B)]
            val_b = [persist.tile([128, TTI * VBLK], BF16, tag=f"val{b}", name=f"val{b}")
                     for b in range(B)]
            qb_sb = persist.tile([PD, 1], F32)
            kb_sb = persist.tile([PD, 1], F32)
            bp_sb = persist.tile([PD, 1], F32)
            wq_sb = persist.tile([128, KT * PD], BF16)  # k-tiles side by side
            wk_sb = persist.tile([128, KT * PD], BF16)
            wv_sb = persist.tile([128, KT * PD], BF16)
            wp_sb = persist.tile([128, KT * PD], BF16)

            for b in range(B):
                nc.vector.memset(val_b[b][:], 1.0)  # ones cols survive copies
            nc.sync.dma_start(out=qb_sb[:], in_=qbias.ap())
            nc.sync.dma_start(out=kb_sb[:], in_=kbias.ap())
            nc.sync.dma_start(out=bp_sb[:], in_=bpcol.ap())
            for k in range(KT):
                nc.sync.dma_start(out=wk_sb[:, k * PD:(k + 1) * PD],
                                  in_=wk[k * 128:(k + 1) * 128, :])
                nc.sync.dma_start(out=wv_sb[:, k * PD:(k + 1) * PD],
                                  in_=wv[k * 128:(k + 1) * 128, :])
                nc.sync.dma_start(out=wq_sb[:, k * PD:(k + 1) * PD],
                                  in_=wq[k * 128:(k + 1) * 128, :])
                nc.sync.dma_start(out=wp_sb[:, k * PD:(k + 1) * PD],
                                  in_=wp[k * 128:(k + 1) * 128, :])

            # input tiles, split per batch so batch-0 work starts earliest:
            # load order et_b0, xt_b0, et_b1, xt_b1
            eT_t = [[None] * KT for _ in range(B)]
            xT_t = [[None] * KT for _ in range(B)]
            # batch-0 inputs ride the two fast queues (gpsimd/scalar);
            # weights and batch-1 inputs go on the sync queue in parallel
            for b in range(B):
                for k in range(KT):
                    et = persist.tile([128, T], BF16, tag=f"et{b}_{k}",
                                      name=f"et{b}_{k}")
                    eng = (nc.gpsimd if k < 4 else nc.scalar) if b == 0 else nc.sync
                    eng.dma_start(
                        out=et[:], in_=eT[k * 128:(k + 1) * 128,
                                          b * T:(b + 1) * T])
                    eT_t[b][k] = et
                for k in range(KT):
                    xt = persist.tile([128, S], BF16, tag=f"xt{b}_{k}",
                                      name=f"xt{b}_{k}")
                    eng = (nc.gpsimd if k < 4 else nc.scalar) if b == 0 else nc.sync
                    eng.dma_start(
                        out=xt[:], in_=xT[k * 128:(k + 1) * 128,
                                          b * S:(b + 1) * S])
                    xT_t[b][k] = xt

            # DRAM bounce + AllGather buffers, one per (b, s-window)
            av_dram = [[dram.tile([PD, SWW], BF16, tag=f"avd{b}{sw}", name=f"avd{b}{sw}")
                        for sw in range(SW)] for b in range(B)]
            ag_dram = [[dram.tile([HD, SWW], BF16, tag=f"agd{b}{sw}", name=f"agd{b}{sw}",
                                  addr_space="Shared")
                        for sw in range(SW)] for b in range(B)]

            # ---------------- unit generators --------------------------
            def proj_chunk_unit(b, ch, which):
                """512-wide chunk of the qT / kT projection for batch b."""
                def emit():
                    src = xT_t[b] if which == "q" else eT_t[b]
                    w = wq_sb if which == "q" else wk_sb
                    bias = qb_sb if which == "q" else kb_sb
                    dst = qT_b[b] if which == "q" else kT_b[b]
                    p = psum.tile([PD, 512], F32, tag="ps", name="pproj")
                    for k in range(KT):
                        nc.tensor.matmul(p[:], w[:, k * PD:(k + 1) * PD],
                                         src[k][:, ch * 512:(ch + 1) * 512],
                                         start=(k == 0), stop=(k == KT - 1))
                    nc.vector.tensor_scalar_add(
                        dst[:, ch * 512:(ch + 1) * 512], p[:], bias[:])
                return emit

            def val_unit(b, ti):
                """One 128-row t-tile of the val projection for batch b."""
                def emit():
                    p = psum.tile([128, PD], F32, tag="ps", name="pval")
                    for k in range(KT):
                        nc.tensor.matmul(p[:],
                                         eT_t[b][k][:, ti * 128:(ti + 1) * 128],
                                         wv_sb[:, k * PD:(k + 1) * PD],
                                         start=(k == 0), stop=(k == KT - 1))
                    for h in range(H_LOC):
                        # values at block offsets 0..63, ones col at 64
                        nc.vector.tensor_copy(
                            val_b[b][:, ti * VBLK + h * 65:
                                    ti * VBLK + h * 65 + 64],
                            p[:, h * 64:(h + 1) * 64])
                return emit

            def proj_units(b):
                units = []
                for ch in range(4):
                    units.append(proj_chunk_unit(b, ch, "k"))
                for ti in range(TTI):
                    units.append(val_unit(b, ti))
                for ch in range(4):
                    units.append(proj_chunk_unit(b, ch, "q"))
                return units

            def outproj_units(b, sw):
                """Transposed output projection, out^T[e, bs] slice for
                columns b*S + sw*SWW .. +SWW, in two 512-wide chunks."""
                units = []
                ag_tiles = {}

                def load_unit(ch):
                    def emit():
                        for k in range(KT):
                            a = agp.tile([128, 512], BF16, tag="ag", name="ag")
                            nc.sync.dma_start(
                                out=a[:],
                                in_=ag_dram[b][sw][k * 128:(k + 1) * 128,
                                                   ch * 512:(ch + 1) * 512])
                            ag_tiles[(ch, k)] = a
                    return emit

                def mm_unit(ch):
                    def emit():
                        po = psum.tile([128, 512], F32, tag="ps", name="po")
                        for k in range(KT):
                            nc.tensor.matmul(
                                po[:], wp_sb[:, k * PD:(k + 1) * PD],
                                ag_tiles[(ch, k)][:],
                                start=(k == 0), stop=(k == KT - 1))
                        o = otp.tile([128, 512], F32, tag="o", name="o")
                        nc.vector.tensor_scalar_add(o[:], po[:], bp_sb[:])
                        col = b * S + sw * SWW + ch * 512
                        nc.sync.dma_start(out=out[:, col:col + 512], in_=o[:])
                    return emit
                for ch in range(2):
                    units.append(load_unit(ch))
                    units.append(mm_unit(ch))
                return units

            # ---------------- attention for one batch ------------------
            def attention(b, filler, fill_from=0, early_double=0):
                """Attention for batch b; pops one filler unit per (sw, ti)
                iteration index >= fill_from (two per iteration for the first
                `early_double` iterations) to keep TensorE busy."""
                it = 0
                for sw in range(SW):
                    avp = [psum.tile([65, SWW], F32, tag="av", name="avp") for _ in range(H_LOC)]
                    for ti in range(TTI):
                        psc = [psum.tile([128, SWW], F32, tag="ps", name="psc")
                               for _ in range(H_LOC)]
                        # scores: interleave heads so the K=64 matmuls pair up
                        # in the PE array (row groups 0-63 / 64-127)
                        for n in range(2):
                            for h in range(H_LOC):
                                hsl = slice(h * 64, (h + 1) * 64)
                                nc.tensor.matmul(
                                    psc[h][:, n * 512:(n + 1) * 512],
                                    kT_b[b][hsl, ti * 128:(ti + 1) * 128],
                                    qT_b[b][hsl, sw * SWW + n * 512:
                                            sw * SWW + (n + 1) * 512],
                                    start=True, stop=True)
                        for h in range(H_LOC):
                            p_sb = att.tile([128, SWW], BF16, tag="p", name="p_sb")
                            nc.scalar.activation(
                                p_sb[:], psc[h][:],
                                mybir.ActivationFunctionType.Exp, scale=SCALE)
                            vblk = val_b[b][:, ti * VBLK + h * 65:
                                            ti * VBLK + h * 65 + 65]
                            for n in range(2):
                                nc.tensor.matmul(
                                    avp[h][:, n * 512:(n + 1) * 512],
                                    vblk, p_sb[:, n * 512:(n + 1) * 512],
                                    start=(ti == 0), stop=(ti == TTI - 1))
                        if filler and it >= fill_from and \
                                (it >= 10 or it % 2 == 0):
                            filler.popleft()()
                            if filler and it < early_double:
                                filler.popleft()()
                        it += 1
                    # normalize + evict this s-window (sumexp in row 64)
                    av_st = avs.tile([D, 2 * SWW], BF16, tag="avst", name="av_st")
                    for h in range(H_LOC):
                        z = nrm.tile([1, SWW], F32, tag="zrb", name="z")
                        nc.vector.tensor_copy(z[0:1, :], avp[h][64:65, :])
                        zr = nrm.tile([1, SWW], F32, tag="zrb", name="zr")
                        nc.vector.reciprocal_approx_fast(zr[0:1, :], z[0:1, :])
                        rb = nrm.tile([D, SWW], F32, tag="zrb", name="rb")
                        nc.gpsimd.partition_broadcast(rb[:], zr[0:1, :])
                        nc.vector.tensor_mul(
                            av_st[:, h * SWW:(h + 1) * SWW],
                            avp[h][0:64, :], rb[:])
                    nc.sync.dma_start(
                        out=av_dram[b][sw].rearrange("(h d) s -> d h s", h=H_LOC),
                        in_=av_st.rearrange("d (h s) -> d h s", h=H_LOC))
                    nc.gpsimd.collective_compute(
                        "AllGather", mybir.AluOpType.bypass,
                        replica_groups=[list(range(N_CORES))],
                        ins=[av_dram[b][sw].opt()], outs=[ag_dram[b][sw].opt()])

            # ---------------- schedule ---------------------------------
            # prefix: all enc-dependent b0 projections run while the x/enc
            # DMAs stream in, then the first two qT chunks
            prefix = [proj_chunk_unit(0, ch, "k") for ch in range(4)] + \
                     [val_unit(0, ti) for ti in range(TTI)] + \
                     [proj_chunk_unit(0, 0, "q"), proj_chunk_unit(0, 1, "q")]
            for u in prefix:
                u()
            fill0 = deque(
                [proj_chunk_unit(0, 2, "q"), proj_chunk_unit(0, 3, "q")] +
                [proj_chunk_unit(1, ch, "k") for ch in range(4)] +
                [val_unit(1, ti) for ti in range(TTI)] +
                [proj_chunk_unit(1, ch, "q") for ch in range(4)])
            attention(0, fill0, fill_from=0)
            for u in fill0:
                u()
            fill1 = deque(outproj_units(0, 0) + outproj_units(0, 1))
            attention(1, fill1, fill_from=8)
            for u in fill1:
                u()
            # tail: outproj b1/sw0 right away, dummies keep PE warm while the
            # last AllGather finishes, then outproj b1/sw1
            for u in outproj_units(1, 0):
                u()
            for i in range(16):
                proj_chunk_unit(0, i % 4, "q")()
            for u in outproj_units(1, 1):
                u()

    nc.compile()
    return nc


_NC_CACHE = None


def _get_program():
    global _NC_CACHE
    if _NC_CACHE is None:
        _NC_CACHE = build_program()
    return _NC_CACHE


def _make_in_maps(inputs, pos_embedding, encoder_hidden_states, u, v, mask,
                  Wkv, bkv, Wq, bq, Wp, bp):
    bf = ml_dtypes.bfloat16
    xT = np.ascontiguousarray(
        np.asarray(inputs, np.float32).transpose(2, 1, 0).reshape(E, BS)).astype(bf)
    eT = np.ascontiguousarray(
        np.asarray(encoder_hidden_states, np.float32).transpose(2, 1, 0)
        .reshape(E, BT)).astype(bf)
    Wkv = np.asarray(Wkv, np.float32)
    Wq = np.asarray(Wq, np.float32)
    Wp = np.asarray(Wp, np.float32)
    bkv = np.asarray(bkv, np.float32)
    bq = np.asarray(bq, np.float32)
    bp = np.asarray(bp, np.float32)
    uf = np.asarray(u, np.float32).reshape(HD)
    in_maps = []
    for c in range(N_CORES):
        sl = slice(c * PD, (c + 1) * PD)
        in_maps.append({
            "xT": xT,
            "eT": eT,
            "wq": np.ascontiguousarray(Wq[:, sl]).astype(bf),
            "wk": np.ascontiguousarray(Wkv[:, sl]).astype(bf),
            "wv": np.ascontiguousarray(Wkv[:, HD + c * PD: HD + (c + 1) * PD]).astype(bf),
            "wp": np.ascontiguousarray(Wp[:, sl]).astype(bf),
            "qbias": (uf[sl] + bq[sl]).reshape(PD, 1).astype(np.float32),
            "kbias": bkv[sl].reshape(PD, 1).astype(np.float32),
            "bpcol": (bp[sl] + bkv[HD:] @ Wp[:, sl]).reshape(PD, 1)
                     .astype(np.float32),
        })
    return in_maps


def _assemble(results):
    full = np.empty((S, B, E), np.float32)
    for c in range(N_CORES):
        part = np.asarray(results[c]["out"]).reshape(PD, B, S)
        full[:, :, c * PD:(c + 1) * PD] = part.transpose(2, 1, 0)
    return full


def run(trace=False, **inputs):
    nc = _get_program()
    in_maps = _make_in_maps(**inputs)
    res = run_bass_kernel_spmd(nc, in_maps, core_ids=list(range(N_CORES)),
                               trace=trace)
    return _assemble(res.results), res


def kernel(**inputs):
    out, _ = run(**inputs)
    return out
